# revision 45
# baseline (speedup 1.0000x reference)
"""GQA attention prefill (Qwen3-style) on 8 TRN2 NeuronCores.

Sharding: core c -> batch b = c // 4, kv-head pair j = c % 4
  (kv heads {2j, 2j+1}, q heads {4j..4j+3}).
Per core: fused QKV projection (fp32r matmuls), per-head RMSNorm + RoPE
(norm weights and the 1/sqrt(HD) score scale are folded into host-side
cos/sin tables), PE transposes into K^T/Q^T layouts, attention with
transposed scores (scoresT[t, s]), exp on the scalar engine into fp16
P tiles, PV + softmax-denominator (ones-matmul) accumulated in PSUM,
per-head 1/l normalization via gpsimd partition_broadcast, fp32r output
projection. Host sums the 4 partial outputs per batch (the o-projection
all-reduce).
"""

import numpy as np
import ml_dtypes

B, S, HID = 2, 1024, 1024
NH, NKV, HD = 16, 8, 128
G = NH // NKV
CACHE_LEN, MAX_CACHE = 3072, 4096
T = CACHE_LEN + S                  # 4096
N_TT = T // 128                    # 32 t-tiles
N_CT = CACHE_LEN // 128            # 24 cached t-tiles
THETA = 1000000.0
EPS = 1e-6

_STATE = {}


def _build():
    import concourse.bass as bass
    import concourse.tile as tile
    from concourse import bacc, mybir

    f32 = mybir.dt.float32
    f32r = mybir.dt.float32r
    f16 = mybir.dt.float16
    i16 = mybir.dt.int16
    # fp16 Schraudolph exp: bits = round(x*1024*log2(e) + (15*1024 - 44.7))
    SCH_A = 1024 * 1.4426950408889634
    SCH_B = 15 * 1024 - 44.7
    AF = mybir.ActivationFunctionType
    OP = mybir.AluOpType

    nc = bacc.Bacc("TRN2", target_bir_lowering=False, debug=False, num_devices=8)

    xt_d = nc.dram_tensor("xt", [128, 8, 1024], f16, kind="ExternalInput").ap()
    wq_d = nc.dram_tensor("wq", [128, 8, 1024], f16, kind="ExternalInput").ap()
    kc_d = nc.dram_tensor("kc", [128, 2, CACHE_LEN], f16, kind="ExternalInput").ap()
    vc_d = nc.dram_tensor("vc", [128, N_CT, 2, 128], f16, kind="ExternalInput").ap()
    cq_d = nc.dram_tensor("cq", [128, 8, 128], f16, kind="ExternalInput").ap()
    sq_d = nc.dram_tensor("sq", [128, 8, 128], f16, kind="ExternalInput").ap()
    ck_d = nc.dram_tensor("ck", [128, 8, 128], f16, kind="ExternalInput").ap()
    sk_d = nc.dram_tensor("sk", [128, 8, 128], f16, kind="ExternalInput").ap()
    wo_d = nc.dram_tensor("wo", [128, 4, 1024], f16, kind="ExternalInput").ap()
    tri_d = nc.dram_tensor("tri", [128, 128], f16, kind="ExternalInput").ap()
    one_d = nc.dram_tensor("one", [128, 1], f16, kind="ExternalInput").ap()
    idn_d = nc.dram_tensor("idn", [128, 128], f16, kind="ExternalInput").ap()
    out_d = nc.dram_tensor("out", [S, HID], f32, kind="ExternalOutput").ap()

    with tile.TileContext(nc) as tc:
        with tc.tile_pool(name="persist", bufs=1) as persist:
            kT = persist.tile([128, 2, T], f16, tag="kT")        # [d, kv, t]
            vT = persist.tile([128, N_TT, 2, 128], f16, tag="vT")  # [tp, ti, kv, d]
            qT = persist.tile([128, 4, S], f16, tag="qT")        # [d, h, s]
            ctx = persist.tile([128, 4, S], f16, tag="ctx")      # [d, h, s]
            wo_sb = persist.tile([128, 4, 1024], f16, tag="wo")
            tri_sb = persist.tile([128, 128], f16, tag="tri")
            one_sb = persist.tile([128, 1], f16, tag="one")
            idn_sb = persist.tile([128, 128], f16, tag="idn")

            # ---------------- Phase 1: QKV projection + norm + rope ----------
            with tc.tile_pool(name="ph1", bufs=1) as ph1, \
                 tc.tile_pool(name="qkp", bufs=3) as qkp, \
                 tc.tile_pool(name="tmp", bufs=2) as tmp, \
                 tc.tile_pool(name="stat", bufs=8) as statp, \
                 tc.tile_pool(name="ps1", bufs=2, space="PSUM") as ps1, \
                 tc.tile_pool(name="pstp", bufs=2, space="PSUM") as pstp:
                xt_sb = ph1.tile([128, 8, 1024], f16, tag="xt")
                wq_sb = ph1.tile([128, 8, 1024], f16, tag="wqkv")
                cq_sb = ph1.tile([128, 8, 128], f16, tag="cq")
                sq_sb = ph1.tile([128, 8, 128], f16, tag="sq")
                ck_sb = ph1.tile([128, 8, 128], f16, tag="ck")
                sk_sb = ph1.tile([128, 8, 128], f16, tag="sk")
                # One explicit ACT table load (set 6 = natural_log_exp_and_
                # others, covers Copy/Ln/Exp/Square) at t=0, overlapped with
                # the input DMAs; bacc's fixpoint pass then sees every
                # activation's table already loaded on all paths.
                eps_t = ph1.tile([128, 1], f32, tag="eps")
                zero_t = ph1.tile([128, 1], f32, tag="zero")
                nc.vector.memset(eps_t[:], EPS)
                nc.vector.memset(zero_t[:], 0.0)
                nc.scalar.add_instruction(mybir.InstLoadActFuncSet(
                    name=nc.get_next_instruction_name(), ins=[], outs=[],
                    act_func_set_id=6))
                # interleave xt/wq per k-tile so the first matmuls start early
                for kt in range(8):
                    nc.sync.dma_start(out=xt_sb[:, kt, :], in_=xt_d[:, kt, :])
                    nc.sync.dma_start(out=wq_sb[:, kt, :], in_=wq_d[:, kt, :])
                nc.sync.dma_start(out=cq_sb[:], in_=cq_d[:])
                nc.sync.dma_start(out=sq_sb[:], in_=sq_d[:])
                nc.sync.dma_start(out=ck_sb[:], in_=ck_d[:])
                nc.sync.dma_start(out=sk_sb[:], in_=sk_d[:])
                nc.sync.dma_start(out=idn_sb[:], in_=idn_d[:])
                for tch in range(4):
                    nc.sync.dma_start(
                        out=kT[:, :, 768 * tch:768 * (tch + 1)],
                        in_=kc_d[:, :, 768 * tch:768 * (tch + 1)])
                nc.sync.dma_start(out=tri_sb[:], in_=tri_d[:])
                nc.sync.dma_start(out=one_sb[:], in_=one_d[:])
                nc.sync.dma_start(out=vT[:, 0:N_CT, :, :], in_=vc_d[:])
                nc.sync.dma_start(out=wo_sb[:], in_=wo_d[:])

                # 4-stage software pipeline over m: each engine's stream
                # always has ready work (strict per-engine program order).
                stA = {}

                def stage_a(m):  # PE matmuls + ACT stats (reads ps only)
                    ps = ps1.tile([128, 1024], f32, tag="qkvps",
                                  name=f"qkvps{m}")
                    for c in range(2):
                        for kt in range(8):
                            nc.tensor.matmul(
                                ps[:, 512 * c:512 * c + 512],
                                lhsT=xt_sb[:, kt, 128 * m:128 * m + 128],
                                rhs=wq_sb[:, kt, 512 * c:512 * c + 512],
                                start=(kt == 0),
                                stop=(kt == 7),
                            )
                    nc.vector.tensor_copy(
                        out=vT[:, N_CT + m, :, :],
                        in_=ps[:, 768:1024].rearrange("p (a b) -> p a b", a=2),
                    )
                    sqj = tmp.tile([128, 128], f32, tag="sqj",
                                   name=f"sqj{m}")
                    rstd = statp.tile([128, 6], f32, tag="rstd",
                                      name=f"rstd{m}")
                    for hi in range(6):
                        nc.scalar.activation(
                            out=sqj[:], in_=ps[:, 128 * hi:128 * hi + 128],
                            func=AF.Square, accum_out=rstd[:, hi:hi + 1],
                        )
                    # rstd = (ms + eps)^-0.5 = exp(-0.5 * ln(ms + eps))
                    nc.scalar.activation(
                        out=rstd[:], in_=rstd[:], func=AF.Ln,
                        bias=eps_t[:], scale=1.0 / HD,
                    )
                    nc.scalar.activation(
                        out=rstd[:], in_=rstd[:], func=AF.Exp,
                        bias=zero_t[:], scale=-0.5,
                    )
                    stA[m] = (ps, rstd)

                def stage_b(m):  # DVE: normalized q/k copies out of PSUM
                    ps, rstd = stA[m]
                    qn = qkp.tile([128, 768], f32, tag="qk", name=f"qn{m}")
                    for hi in range(6):
                        sl = slice(128 * hi, 128 * hi + 128)
                        nc.vector.tensor_scalar_mul(
                            out=qn[:, sl], in0=ps[:, sl],
                            scalar1=rstd[:, hi:hi + 1],
                        )
                    stA[m] = qn

                def stage_c(m):  # DVE/Pool: rope
                    qn = stA[m]
                    qn4 = qn[:, 0:512].rearrange("p (h d) -> p h d", h=4)
                    qn2 = qn[:, 512:768].rearrange("p (h d) -> p h d", h=2)
                    t1 = tmp.tile([128, 768], f16, tag="t1", name=f"t1_{m}")
                    t2 = tmp.tile([128, 768], f16, tag="t2", name=f"t2_{m}")
                    t1q = t1[:, 0:512].rearrange("p (h d) -> p h d", h=4)
                    t1k = t1[:, 512:768].rearrange("p (h d) -> p h d", h=2)
                    t2q = t2[:, 0:512].rearrange("p (h d) -> p h d", h=4)
                    t2k = t2[:, 512:768].rearrange("p (h d) -> p h d", h=2)
                    cqb = cq_sb[:, m, :].unsqueeze(1).broadcast_to((128, 4, 128))
                    ckb = ck_sb[:, m, :].unsqueeze(1).broadcast_to((128, 2, 128))
                    sqb = sq_sb[:, m, :].unsqueeze(1).broadcast_to((128, 4, 128))
                    skb = sk_sb[:, m, :].unsqueeze(1).broadcast_to((128, 2, 128))
                    nc.vector.tensor_mul(t1q, qn4, cqb)
                    nc.vector.tensor_mul(t1k, qn2, ckb)
                    nc.vector.tensor_mul(
                        t2q[:, :, 0:64], qn4[:, :, 64:128], sqb[:, :, 0:64])
                    nc.vector.tensor_mul(
                        t2q[:, :, 64:128], qn4[:, :, 0:64], sqb[:, :, 64:128])
                    nc.vector.tensor_mul(
                        t2k[:, :, 0:64], qn2[:, :, 64:128], skb[:, :, 0:64])
                    nc.vector.tensor_mul(
                        t2k[:, :, 64:128], qn2[:, :, 0:64], skb[:, :, 64:128])
                    nc.gpsimd.tensor_add(t1[:], t1[:], t2[:])
                    stA[m] = t1

                def stage_d(m):  # PE transposes + copies into qT/kT
                    t1 = stA.pop(m)
                    tp = pstp.tile([128, 768], f16, tag="tp", name=f"tp{m}")
                    for hi in range(6):
                        nc.tensor.transpose(
                            tp[:, 128 * hi:128 * hi + 128],
                            t1[:, 128 * hi:128 * hi + 128], idn_sb[:])
                    # first half on DVE: shortens the ACT queue ahead of the
                    # first attention tiles (qT for head 0 chunk 0)
                    cp = nc.vector.tensor_copy if m < 4 else nc.scalar.copy
                    cp(out=qT[:, :, 128 * m:128 * m + 128],
                       in_=tp[:, 0:512].rearrange("p (h d) -> p h d", h=4))
                    cp(out=kT[:, :, CACHE_LEN + 128 * m:
                              CACHE_LEN + 128 * m + 128],
                       in_=tp[:, 512:768].rearrange("p (h d) -> p h d", h=2))

                for step in range(10):
                    if step >= 2:
                        stage_d(step - 2)
                    if 1 <= step <= 8:
                        stage_b(step - 1)
                        stage_c(step - 1)
                    if step < 8:
                        stage_a(step)

            # ---------------- Phase 2: attention ----------------------------
            with tc.tile_pool(name="pp", bufs=8) as ppool, \
                 tc.tile_pool(name="bcp", bufs=2) as bcp, \
                 tc.tile_pool(name="ltree", bufs=3) as ltree, \
                 tc.tile_pool(name="sps", bufs=2, space="PSUM") as sps, \
                 tc.tile_pool(name="lrp", bufs=1, space="PSUM") as lrp, \
                 tc.tile_pool(name="cps", bufs=1, space="PSUM") as cps:
                tail_jobs = []
                for h in range(4):
                    kv = h // 2
                    ctx_ps = cps.tile([128, S], f32, tag="ctxps",
                                      name=f"ctxps{h}")
                    # l-root lives in its own PSUM pool so the scores ring
                    # never waits on the previous head's reciprocal
                    l_ps = lrp.tile([1, S], f32, tag="lroot", name=f"lroot{h}")

                    def pv(i, s_lo, P_t):
                        for c in range(2):
                            c_lo, c_hi = max(s_lo, 512 * c), 512 * (c + 1)
                            if c_lo >= c_hi:
                                continue
                            last_i = N_CT + 4 * (c + 1) - 1
                            nc.tensor.matmul(
                                ctx_ps[:, c_lo:c_hi],
                                lhsT=vT[:, i, kv, :],
                                rhs=P_t[:, c_lo:c_hi],
                                start=(i == 0), stop=(i == last_i),
                            )

                    # online pairwise tree for l[s] = sum_t P[t, s]: fp16
                    # adds on DVE (2x mode) replace a PE ones-matmul stream
                    levels = [None] * 6
                    pend = []
                    for i in range(N_TT):
                        s_lo = max(0, 128 * (i - N_CT))
                        # PV trails so PE fills the exp latency with the
                        # next tiles' QK matmuls.
                        if len(pend) > (4 if i < 8 else 2):
                            pv(*pend.pop(0))
                        if 1 <= i:
                            for _ in range(3):
                                if tail_jobs:
                                    tail_jobs.pop(0)()
                        P_t = ppool.tile([128, S], f16, tag="P")
                        sc = sps.tile([128, S], f32, tag="sc", name=f"sc{h}_{i}")
                        for c in range(2):
                            c_lo, c_hi = max(s_lo, 512 * c), 512 * (c + 1)
                            if c_lo >= c_hi:
                                continue
                            nc.tensor.matmul(
                                sc[:, c_lo:c_hi],
                                lhsT=kT[:, kv, 128 * i:128 * i + 128],
                                rhs=qT[:, h, c_lo:c_hi],
                                start=True, stop=True,
                            )
                        nc.scalar.activation(
                            out=P_t[:, s_lo:S], in_=sc[:, s_lo:S],
                            func=AF.Exp,
                        )
                        if i >= N_CT:
                            nc.gpsimd.tensor_mul(
                                P_t[:, s_lo:s_lo + 128],
                                P_t[:, s_lo:s_lo + 128],
                                tri_sb[:],
                            )
                            if s_lo > 0:
                                nc.gpsimd.memset(P_t[:, 0:s_lo], 0.0)
                        pend.append((i, s_lo, P_t))
                        cur, k = P_t, 0
                        while levels[k] is not None:
                            nxt = ltree.tile([128, S], f16, tag=f"lv{k + 1}",
                                             name=f"lv{k + 1}_{h}_{i}")
                            nc.vector.tensor_add(nxt[:], levels[k][:], cur[:])
                            levels[k] = None
                            cur, k = nxt, k + 1
                        levels[k] = cur
                    for job in pend:
                        pv(*job)

                    def make_tail(h, kv, ctx_ps, root, l_ps=l_ps):
                        def t_root():
                            for c in range(2):
                                nc.tensor.matmul(
                                    l_ps[:, 512 * c:512 * c + 512],
                                    lhsT=one_sb[:],
                                    rhs=root[:, 512 * c:512 * c + 512],
                                    start=True, stop=True,
                                )
                            rl = bcp.tile([1, S], f32, tag="rl",
                                          name=f"rl{h}")
                            nc.vector.reciprocal_approx_fast(
                                out=rl[:], in_=l_ps[:])
                            bc = bcp.tile([128, S], f32, tag="bc",
                                          name=f"bc{h}")
                            nc.gpsimd.partition_broadcast(
                                out_ap=bc[:], in_ap=rl[:])
                            make_tail.bc = bc
                        def t_mul(c4):
                            def run():
                                sl = slice(256 * c4, 256 * c4 + 256)
                                nc.vector.tensor_mul(
                                    ctx[:, h, sl], ctx_ps[:, sl],
                                    make_tail.bc[:, sl])
                            return run
                        return [t_root] + [t_mul(c4) for c4 in range(4)]

                    tail_jobs.extend(make_tail(h, kv, ctx_ps, levels[5]))
                    if h == 3:
                        while tail_jobs:
                            tail_jobs.pop(0)()

            # ---------------- Phase 3: output projection --------------------
            with tc.tile_pool(name="osb", bufs=3) as osb, \
                 tc.tile_pool(name="ops", bufs=3, space="PSUM") as ops:
                for m in range(8):
                    op = ops.tile([128, 1024], f32, tag="ops", name=f"op{m}")
                    for c2 in range(2):
                        for h2 in range(4):
                            nc.tensor.matmul(
                                op[:, 512 * c2:512 * c2 + 512],
                                lhsT=ctx[:, h2, 128 * m:128 * m + 128],
                                rhs=wo_sb[:, h2, 512 * c2:512 * c2 + 512],
                                start=(h2 == 0), stop=(h2 == 3),
                            )
                    ot = osb.tile([128, 1024], f32, tag="ot", name=f"ot{m}")
                    nc.scalar.copy(out=ot[:], in_=op[:])
                    nc.sync.dma_start(
                        out=out_d[128 * m:128 * m + 128, :], in_=ot[:])

    nc.compile()
    return nc


def _get_nc():
    if "nc" not in _STATE:
        _STATE["nc"] = _build()
    return _STATE["nc"]


def _host_tables(q_norm_w, k_norm_w, cache_len):
    pos = np.arange(cache_len, cache_len + S, dtype=np.float32)
    inv_freq = (1.0 / (THETA ** (np.arange(0, HD, 2, dtype=np.float32) / HD))) \
        .astype(np.float32)
    freqs = pos[:, None] * inv_freq[None, :]          # [S, 64]
    emb = np.concatenate([freqs, freqs], axis=-1)     # [S, HD]
    cos = np.cos(emb).astype(np.float32)
    sin = np.sin(emb).astype(np.float32)

    qs = np.float32(HD ** -0.5)
    cq = cos * q_norm_w[None, :] * qs
    ck = cos * k_norm_w[None, :]
    # rotate_half coefficient tables: out[d<64] += x[d+64] * (-sin[d] * w[d+64])
    #                                 out[d>=64] += x[d-64] * (sin[d] * w[d-64])
    sq = np.empty_like(sin)
    sq[:, :64] = -sin[:, :64] * q_norm_w[None, 64:]
    sq[:, 64:] = sin[:, 64:] * q_norm_w[None, :64]
    sq = sq * qs
    sk = np.empty_like(sin)
    sk[:, :64] = -sin[:, :64] * k_norm_w[None, 64:]
    sk[:, 64:] = sin[:, 64:] * k_norm_w[None, :64]

    def tile8(a):  # [S, 128] -> [128, 8, 128]
        return np.ascontiguousarray(
            a.reshape(8, 128, 128).transpose(1, 0, 2)
        ).astype(np.float16)

    return tile8(cq), tile8(sq), tile8(ck), tile8(sk)


def kernel(hidden_states, qkv_weight, q_norm_w, k_norm_w, o_weight,
           k_cache, v_cache, cache_len):
    from concourse.bass_utils import run_bass_kernel_spmd

    assert int(cache_len) == CACHE_LEN, "kernel compiled for cache_len=3072"
    hs = np.asarray(hidden_states, dtype=np.float32)
    wqkv = np.asarray(qkv_weight, dtype=np.float32)
    qnw = np.asarray(q_norm_w, dtype=np.float32)
    knw = np.asarray(k_norm_w, dtype=np.float32)
    wo = np.asarray(o_weight, dtype=np.float32)
    kc = np.asarray(k_cache, dtype=np.float32)
    vc = np.asarray(v_cache, dtype=np.float32)

    cq, sq, ck, sk = _host_tables(qnw, knw, int(cache_len))
    tri = np.triu(np.ones((128, 128), np.float32)).astype(np.float16)
    one = np.ones((128, 1), np.float16)
    idn = np.eye(128, dtype=np.float16)

    in_maps = []
    for c in range(8):
        b, j = c // 4, c % 4
        xt = np.ascontiguousarray(
            hs[b].T.reshape(8, 128, S).transpose(1, 0, 2)).astype(np.float16)
        wrows = np.concatenate([
            wqkv[512 * j:512 * j + 512],
            wqkv[2048 + 256 * j:2048 + 256 * j + 256],
            wqkv[3072 + 256 * j:3072 + 256 * j + 256],
        ], axis=0)                                     # [1024, HID]
        wq = np.ascontiguousarray(
            wrows.T.reshape(8, 128, 1024).transpose(1, 0, 2)).astype(np.float16)
        kcc = np.ascontiguousarray(
            kc[b, :CACHE_LEN, 2 * j:2 * j + 2, :].transpose(2, 1, 0)
        ).astype(np.float16)
        vcc = np.ascontiguousarray(
            vc[b, :CACHE_LEN, 2 * j:2 * j + 2, :]
            .reshape(N_CT, 128, 2, 128).transpose(1, 0, 2, 3)
        ).astype(np.float16)
        wot = np.ascontiguousarray(
            wo[:, 512 * j:512 * j + 512].T.reshape(4, 128, 1024)
            .transpose(1, 0, 2)).astype(np.float16)
        in_maps.append({
            "xt": xt, "wq": wq, "kc": kcc, "vc": vcc,
            "cq": cq, "sq": sq, "ck": ck, "sk": sk,
            "wo": wot, "tri": tri, "one": one, "idn": idn,
        })

    nc = _get_nc()
    _STATE["last_in_maps"] = in_maps
    res = run_bass_kernel_spmd(nc, in_maps, core_ids=list(range(8)))
    outs = [res.results[i]["out"] for i in range(8)]
    full = np.empty((B, S, HID), np.float32)
    for b in range(B):
        full[b] = outs[4 * b] + outs[4 * b + 1] + outs[4 * b + 2] + outs[4 * b + 3]
    return full
